# revision 10
# baseline (speedup 1.0000x reference)
"""Trainium2 Bass kernel for nn_L1OutUB (L1-out upper bound contrastive loss).

Math: the reference builds a [B,B,B] tensor `inpt[a,i,j] = all_probs[i,j] +
(-20 if a==i else 0)` and logsumexps over `a`.  That logsumexp is exactly
`all_probs[i,j] + log(B-1+e^-20)`, so

    result = mean(positive) - mean(all_probs) - log1p(e^-20 / (B-1))

and `sum_j all_probs[i,j]` collapses onto per-column moments of y
(S2[d] = sum_j y[j,d]^2, M1[d] = sum_j y[j,d]).  The -0.5*logvar terms
cancel exactly between positive and negative.  Expanding further, the
per-(i,d) mu^2 terms cancel between the positive and all-pairs branches:

    contrib[i,d] = inv[i,d] * ( mu[i,d]*G[i,d] + K[i,d] )
      G = yc/B - M1/B^2          (yc = matched y row, feature-major)
      K = S2/(2B^2) - yc^2/(2B)
      inv = exp(-tanh(z_lv))

Sharding: rows of x across 8 cores (64 rows each); every core gets the full
y (column-rotated so its matched rows sit at cols 0:64 of yT) and computes
the global column moments redundantly.  Host sums the 8 per-partition
partials (the "all-reduce").

Key layout decisions (all transposes done on host, PE does only matmuls):
  - x is shipped pre-transposed as 6 [128,64] feature chunks packed next to
    the L1 weights; y pre-transposed as yT [128, 512] so the moments are
    free-dim DVE reductions and yc is just yT[:, 0:64].
  - L1 runs both nets in one 6-matmul chain ([128,40] stationaries, mu at
    out-partitions 0:8, lv at 32:40 to satisfy the base-partition rule).
    Bias+relu fused into one tensor_scalar(add,max) per net (DVE + GPSIMD).
  - L2 folds the biases via all-ones rows (memset) in the moving operand
    and a bias row in the stationary, so mu/z_lv leave PSUM finished.
  - ACT engine does only tanh and exp (same table set, single load that
    overlaps the DMAs).
  - Final reduce is one fused tensor_tensor_reduce with per-partition
    accumulate; the [128,1] partial sums are DMA'd out and summed on host.
"""

import numpy as np

import concourse.bacc as bacc
import concourse.tile as tile
from concourse import mybir

F32 = mybir.dt.float32
AF = mybir.ActivationFunctionType
ALU = mybir.AluOpType

B, X_DIM, Y_DIM, HID = 512, 768, 128, 8
N_CORES = 8
R = B // N_CORES          # rows per core = 64
XC = X_DIM // 128         # x feature chunks = 6

# blobA [128, A_COLS]: w1 chunks ([128,6,40]: mu at +0:8, lv at +32:40,
#   rest zero), b1 col (rows 0:8 b1_mu, rows 32:40 b1_lv), xT chunks 0:2.
W1C = 41
A_W1 = XC * W1C           # 240
A_B1 = A_W1               # col 240
A_XT = A_B1 + 1           # 241
A_COLS = A_XT + 3 * R     # 433
# blobB1 [128, B1_COLS]: w2 block (rows 0:9 = w2_mu+b2_mu, rows 32:41 =
#   w2_lv+b2_lv) in cols 0:128, xT chunk 3 in cols 128:192.
B1_COLS = Y_DIM + R       # 192
# blobB2 [128, 2*R]: xT chunks 4,5.
B2_COLS = 2 * R           # 128

_CACHE = {}


def _build():
    nc = bacc.Bacc("TRN2", target_bir_lowering=False, debug=False,
                   num_devices=N_CORES)

    ba_d = nc.dram_tensor("ba", [128, A_COLS], F32, kind="ExternalInput")
    bb1_d = nc.dram_tensor("bb1", [128, B1_COLS], F32, kind="ExternalInput")
    bb2_d = nc.dram_tensor("bb2", [128, B2_COLS], F32, kind="ExternalInput")
    yt_d = nc.dram_tensor("yt", [128, B], F32, kind="ExternalInput")
    out_d = nc.dram_tensor("out", [128, 1], F32, kind="ExternalOutput")

    with tile.TileContext(nc) as tc:
        with (
            tc.tile_pool(name="sb", bufs=1) as sb,
            tc.tile_pool(name="ps", bufs=1, space="PSUM") as ps,
        ):
            # ---- loads: blobA on SP queue, blobB halves on ACT queue,
            # yT via SWDGE so both HWDGE rings stay on the x path.
            ba_s = sb.tile([128, A_COLS], F32, tag="ba")
            nc.sync.dma_start(out=ba_s[:], in_=ba_d[:])
            bb1_s = sb.tile([128, B1_COLS], F32, tag="bb1")
            nc.scalar.dma_start(out=bb1_s[:], in_=bb1_d[:])
            bb2_s = sb.tile([128, B2_COLS], F32, tag="bb2")
            nc.scalar.dma_start(out=bb2_s[:], in_=bb2_d[:])
            yt_s = sb.tile([128, B], F32, tag="yt")
            nc.sync.dma_start(out=yt_s[:, 0:B // 2], in_=yt_d[:, 0:B // 2])
            nc.scalar.dma_start(out=yt_s[:, B // 2:B], in_=yt_d[:, B // 2:B])

            # hb rows 8 and 40 (the L2 all-ones bias rows) come out of
            # relu(0 + 1.0) via the bias column; no memsets needed.
            hb_s = sb.tile([41, R], F32, tag="hbs")

            # ---- MLP layer 1, both nets in one accumulation chain.
            # Chunk order matches expected DMA arrival (blobB first).
            hb_p = ps.tile([W1C, R], F32, tag="hb")
            xt_views = [
                ba_s[:, A_XT:A_XT + R],
                ba_s[:, A_XT + R:A_XT + 2 * R],
                ba_s[:, A_XT + 2 * R:A_XT + 3 * R],
                bb1_s[:, Y_DIM:Y_DIM + R],
                bb2_s[:, 0:R],
                bb2_s[:, R:2 * R],
            ]
            order = [3, 4, 5, 0, 1, 2]
            for i, k in enumerate(order):
                nc.tensor.matmul(hb_p[:], ba_s[:, k * W1C:(k + 1) * W1C],
                                 xt_views[k],
                                 start=(i == 0), stop=(i == len(order) - 1))

            # ---- fused bias+relu for both nets in one DVE op; rows 8:32
            # come out as relu(0 + bias) with bias[8]=1.0 -> the mu ones-row.
            nc.vector.tensor_scalar(out=hb_s[0:W1C, :], in0=hb_p[:],
                                    scalar1=ba_s[0:W1C, A_B1:A_B1 + 1],
                                    scalar2=0.0, op0=ALU.add, op1=ALU.max)

            # ---- MLP layer 2 (bias via ones-row): mu, z_lv in PSUM ----
            mu_p = ps.tile([Y_DIM, R], F32, tag="mup")
            lv_p = ps.tile([Y_DIM, R], F32, tag="lvp")
            nc.tensor.matmul(mu_p[:], bb1_s[0:9, 0:Y_DIM], hb_s[0:9, :],
                             start=True, stop=True)
            nc.tensor.matmul(lv_p[:], bb1_s[32:41, 0:Y_DIM], hb_s[32:41, :],
                             start=True, stop=True)

            # ---- inv = exp(-tanh(z_lv)) on ACT (one table set) ----
            lv_s = sb.tile([Y_DIM, R], F32, tag="lvs")
            nc.scalar.activation(out=lv_s[:], in_=lv_p[:], func=AF.Tanh)
            inv_s = sb.tile([Y_DIM, R], F32, tag="invs")
            nc.scalar.activation(out=inv_s[:], in_=lv_s[:], func=AF.Exp,
                                 scale=-1.0)

            # ---- y column moments on DVE: one fused square+reduce for S2,
            # one tensor_scalar reduce for M1.  ysq cols 0:64 double as yc^2.
            ysq_s = sb.tile([128, B], F32, tag="ysq")
            mom_s = sb.tile([128, 2], F32, tag="mom")
            nc.vector.tensor_mul(ysq_s[:], yt_s[:], yt_s[:])
            nc.vector.tensor_reduce(out=mom_s[:, 1:2], in_=ysq_s[:],
                                    axis=mybir.AxisListType.X, op=ALU.add)
            nc.vector.tensor_reduce(out=mom_s[:, 0:1], in_=yt_s[:],
                                    axis=mybir.AxisListType.X, op=ALU.add)

            # ---- G = yc*B - M1 (raw; 1/B^2 folded later), K from ysq ----
            s2c_s = sb.tile([128, 1], F32, tag="s2c")
            nc.vector.tensor_scalar_mul(s2c_s[:], mom_s[:, 1:2],
                                        0.5 / (B * B))
            g_s = sb.tile([128, R], F32, tag="gs")
            nc.vector.tensor_scalar(out=g_s[:], in0=yt_s[:, 0:R],
                                    scalar1=float(B), scalar2=mom_s[:, 0:1],
                                    op0=ALU.mult, op1=ALU.subtract)
            k_s = sb.tile([128, R], F32, tag="ks")
            nc.vector.tensor_scalar(out=k_s[:], in0=ysq_s[:, 0:R],
                                    scalar1=-0.5 / B, scalar2=s2c_s[:],
                                    op0=ALU.mult, op1=ALU.add)

            # ---- tail: t = mu*G ; q = t/B^2 + K ; r = q*inv (+accum) ----
            t_s = sb.tile([Y_DIM, R], F32, tag="ts")
            nc.vector.tensor_mul(t_s[:], mu_p[:], g_s[:])
            ts_s = sb.tile([Y_DIM, R], F32, tag="tss")
            nc.vector.tensor_scalar_mul(ts_s[:], t_s[:], 1.0 / (B * B))
            q_s = sb.tile([Y_DIM, R], F32, tag="qs")
            nc.vector.tensor_add(q_s[:], ts_s[:], k_s[:])
            w_s = sb.tile([Y_DIM, R], F32, tag="ws")
            nc.vector.tensor_mul(w_s[:], q_s[:], inv_s[:])
            tot_s = sb.tile([128, 1], F32, tag="tot")
            nc.vector.tensor_reduce(out=tot_s[:], in_=w_s[:],
                                    axis=mybir.AxisListType.X, op=ALU.add)
            nc.sync.dma_start(out=out_d[:], in_=tot_s[:])

    nc.compile()
    return nc


def _get_nc():
    if "nc" not in _CACHE:
        _CACHE["nc"] = _build()
    return _CACHE["nc"]


def _pack_weights(w1_mu, b1_mu, w2_mu, b2_mu, w1_lv, b1_lv, w2_lv, b2_lv):
    f = np.float32
    wa = np.zeros((128, A_XT), f)
    w1m = np.asarray(w1_mu, f).reshape(XC, 128, HID)
    w1l = np.asarray(w1_lv, f).reshape(XC, 128, HID)
    for k in range(XC):
        wa[:, k * W1C:k * W1C + 8] = w1m[k]
        wa[:, k * W1C + 32:k * W1C + 40] = w1l[k]
    wa[0:8, A_B1] = np.asarray(b1_mu, f)
    wa[8, A_B1] = 1.0
    wa[32:40, A_B1] = np.asarray(b1_lv, f)
    wa[40, A_B1] = 1.0
    wb = np.zeros((128, Y_DIM), f)
    wb[0:8, :] = np.asarray(w2_mu, f)
    wb[8, :] = np.asarray(b2_mu, f)
    wb[32:40, :] = np.asarray(w2_lv, f)
    wb[40, :] = np.asarray(b2_lv, f)
    return wa, wb


def kernel(x_samples, y_samples, w1_mu, b1_mu, w2_mu, b2_mu,
           w1_lv, b1_lv, w2_lv, b2_lv, **profile_kwargs):
    from concourse import bass_utils

    f = np.float32
    wa, wb = _pack_weights(w1_mu, b1_mu, w2_mu, b2_mu,
                           w1_lv, b1_lv, w2_lv, b2_lv)
    yt = np.ascontiguousarray(np.asarray(y_samples, f).T)      # [128, 512]
    x = np.asarray(x_samples, f)

    in_maps = []
    for c in range(N_CORES):
        xt = np.ascontiguousarray(x[c * R:(c + 1) * R].T)       # [768, 64]
        xt = xt.reshape(XC, 128, R)
        ba = np.empty((128, A_COLS), f)
        ba[:, :A_XT] = wa
        for k in range(3):
            ba[:, A_XT + k * R:A_XT + (k + 1) * R] = xt[k]
        bb1 = np.empty((128, B1_COLS), f)
        bb1[:, :Y_DIM] = wb
        bb1[:, Y_DIM:] = xt[3]
        bb2 = np.empty((128, B2_COLS), f)
        bb2[:, :R] = xt[4]
        bb2[:, R:] = xt[5]
        in_maps.append({
            "ba": ba,
            "bb1": bb1,
            "bb2": bb2,
            "yt": np.ascontiguousarray(np.roll(yt, -c * R, axis=1)),
        })

    nc = _get_nc()
    res = bass_utils.run_bass_kernel_spmd(
        nc, in_maps, core_ids=list(range(N_CORES)), **profile_kwargs
    )
    total = sum(float(m["out"].sum()) for m in res.results)
    total -= np.log1p(np.exp(-20.0) / (B - 1))
    out = np.array(total, dtype=np.float32)
    if profile_kwargs:
        return out, res
    return out


# revision 11
# speedup vs baseline: 1.3302x; 1.3302x over previous
"""Trainium2 Bass kernel for nn_L1OutUB (L1-out upper bound contrastive loss).

Math: the reference builds a [B,B,B] tensor `inpt[a,i,j] = all_probs[i,j] +
(-20 if a==i else 0)` and logsumexps over `a`.  That logsumexp is exactly
`all_probs[i,j] + log(B-1+e^-20)`, so

    result = mean(positive) - mean(all_probs) - log1p(e^-20 / (B-1))

and `sum_j all_probs[i,j]` collapses onto per-column moments of y
(S2[d] = sum_j y[j,d]^2, M1[d] = sum_j y[j,d]).  The -0.5*logvar terms
cancel exactly between positive and negative, and the per-(i,d) mu^2 terms
cancel between the positive and all-pairs branches:

    contrib[i,d] = inv[i,d] * ( mu[i,d]*(yc*B - M1)/B^2 + K[i,d] )
      K   = S2/(2B^2) - yc^2/(2B)     (yc = matched y rows, feature-major)
      inv = exp(-tanh(z_lv))

Sharding: rows of x across 8 cores (64 rows each); every core gets the full
y (column-rotated so its matched rows sit at cols 0:64 of yT) and computes
the global column moments redundantly.  Host sums the 8 scalar partials
(the "all-reduce").

Layout/overlap decisions (all transposes done on host; PE does matmuls only):
  - Two input DMAs per HWDGE queue: blob1 = [w1|b1|xT chunks 0:2|yT half A],
    blob2 = [w2|xT chunks 3:5|yT half B].  x parts stream ahead of y parts;
    y moments are computed per-half as the data lands.
  - y is shipped pre-transposed (yT [128,512]) so moments are free-dim DVE
    reductions and yc/yc^2 are column slices.
  - L1 runs both nets in one 6-matmul chain ([128,41] stationaries, mu rows
    0:8, lv rows 32:40).  Bias+relu fused into one ACT op whose bias column
    also manufactures the two all-ones rows (bias[8]=bias[40]=1, relu(0+1)).
  - L2 folds its biases via those ones-rows, so mu / z_lv leave PSUM done.
  - ACT does relu/tanh/exp only (one table set, load overlaps the DMAs).
  - Final reduce: free-dim DVE reduce -> [128,1], PE matmul against a ones
    column -> [1,1] -> single 4-byte output DMA (a [128,1] output DMA costs
    ~7us in scattered-write completion; don't do that).
"""

import numpy as np

import concourse.bacc as bacc
import concourse.tile as tile
from concourse import mybir

F32 = mybir.dt.float32
AF = mybir.ActivationFunctionType
ALU = mybir.AluOpType

B, X_DIM, Y_DIM, HID = 512, 768, 128, 8
N_CORES = 8
R = B // N_CORES          # rows per core = 64
XC = X_DIM // 128         # x feature chunks = 6
YH = B // 2               # yT half width = 256

W1C = 41                  # L1 stationary cols (mu 0:8, lv 32:40, 40 = ones)
A_W1 = XC * W1C           # 246
A_B1 = A_W1               # bias column index
A_XT = A_B1 + 1           # 247
A_YT = A_XT + 3 * R       # 439
B1_COLS = A_YT + YH       # blob1 width: 695
B_W2 = Y_DIM              # blob2: w2 block cols 0:128
B_XT = B_W2               # xT chunks 3:5 at 128:320
B_YT = B_XT + 3 * R       # 320
B2_COLS = B_YT + YH       # blob2 width: 576

_CACHE = {}


def _build():
    nc = bacc.Bacc("TRN2", target_bir_lowering=False, debug=False,
                   num_devices=N_CORES)

    b1_d = nc.dram_tensor("b1", [128, B1_COLS], F32, kind="ExternalInput")
    b2_d = nc.dram_tensor("b2", [128, B2_COLS], F32, kind="ExternalInput")
    out_d = nc.dram_tensor("out", [1, 1], F32, kind="ExternalOutput")

    with tile.TileContext(nc) as tc:
        with (
            tc.tile_pool(name="sb", bufs=1) as sb,
            tc.tile_pool(name="ps", bufs=1, space="PSUM") as ps,
        ):
            b1_s = sb.tile([128, B1_COLS], F32, tag="b1")
            nc.sync.dma_start(out=b1_s[:], in_=b1_d[:])
            b2_s = sb.tile([128, B2_COLS], F32, tag="b2")
            nc.scalar.dma_start(out=b2_s[:], in_=b2_d[:])

            ones_s = sb.tile([128, 1], F32, tag="ones")
            nc.vector.memset(ones_s[:], 1.0)

            yA = b1_s[:, A_YT:A_YT + YH]
            yB = b2_s[:, B_YT:B_YT + YH]

            # ---- y column moments, one half per blob as it lands ----
            ysq_s = sb.tile([128, YH], F32, tag="ysq")   # only half A kept
            momh_s = sb.tile([128, 4], F32, tag="momh")
            nc.vector.tensor_mul(ysq_s[:], yA, yA)
            nc.vector.tensor_reduce(out=momh_s[:, 0:1], in_=ysq_s[:],
                                    axis=mybir.AxisListType.X, op=ALU.add)
            nc.vector.tensor_reduce(out=momh_s[:, 1:2], in_=yA,
                                    axis=mybir.AxisListType.X, op=ALU.add)

            # ---- MLP layer 1, both nets in one accumulation chain ----
            hb_p = ps.tile([W1C, R], F32, tag="hb")
            xt_views = [
                b1_s[:, A_XT:A_XT + R],
                b1_s[:, A_XT + R:A_XT + 2 * R],
                b1_s[:, A_XT + 2 * R:A_XT + 3 * R],
                b2_s[:, B_XT:B_XT + R],
                b2_s[:, B_XT + R:B_XT + 2 * R],
                b2_s[:, B_XT + 2 * R:B_XT + 3 * R],
            ]
            order = [3, 4, 5, 0, 1, 2]
            for i, k in enumerate(order):
                nc.tensor.matmul(hb_p[:], b1_s[:, k * W1C:(k + 1) * W1C],
                                 xt_views[k],
                                 start=(i == 0), stop=(i == len(order) - 1))

            # ---- fused bias+relu on ACT; rows 8/40 become ones-rows ----
            hb_s = sb.tile([W1C, R], F32, tag="hbs")
            nc.scalar.activation(out=hb_s[:], in_=hb_p[:], func=AF.Relu,
                                 bias=b1_s[0:W1C, A_B1:A_B1 + 1])

            # ---- second-half moments (may land after L1 kicks off) ----
            ysqB_s = sb.tile([128, YH], F32, tag="ysqB")
            nc.vector.tensor_mul(ysqB_s[:], yB, yB)
            nc.vector.tensor_reduce(out=momh_s[:, 2:3], in_=ysqB_s[:],
                                    axis=mybir.AxisListType.X, op=ALU.add)
            nc.vector.tensor_reduce(out=momh_s[:, 3:4], in_=yB,
                                    axis=mybir.AxisListType.X, op=ALU.add)

            # ---- MLP layer 2 (bias via ones-rows): mu, z_lv in PSUM ----
            mu_p = ps.tile([Y_DIM, R], F32, tag="mup")
            lv_p = ps.tile([Y_DIM, R], F32, tag="lvp")
            nc.tensor.matmul(mu_p[:], b2_s[0:9, 0:Y_DIM], hb_s[0:9, :],
                             start=True, stop=True)
            nc.tensor.matmul(lv_p[:], b2_s[32:41, 0:Y_DIM], hb_s[32:41, :],
                             start=True, stop=True)

            # ---- inv = exp(-tanh(z_lv)) on ACT ----
            lv_s = sb.tile([Y_DIM, R], F32, tag="lvs")
            nc.scalar.activation(out=lv_s[:], in_=lv_p[:], func=AF.Tanh)
            inv_s = sb.tile([Y_DIM, R], F32, tag="invs")
            nc.scalar.activation(out=inv_s[:], in_=lv_s[:], func=AF.Exp,
                                 scale=-1.0)

            # ---- combine half-moments; G = yc*B - M1 ; K from ysq ----
            m1_s = sb.tile([128, 1], F32, tag="m1")
            nc.vector.tensor_scalar(out=m1_s[:], in0=momh_s[:, 1:2],
                                    scalar1=momh_s[:, 3:4], scalar2=None,
                                    op0=ALU.add)
            s2c_s = sb.tile([128, 1], F32, tag="s2c")
            nc.vector.tensor_scalar(out=s2c_s[:], in0=momh_s[:, 0:1],
                                    scalar1=momh_s[:, 2:3],
                                    scalar2=0.5 / (B * B),
                                    op0=ALU.add, op1=ALU.mult)
            g_s = sb.tile([128, R], F32, tag="gs")
            nc.vector.tensor_scalar(out=g_s[:], in0=b1_s[:, A_YT:A_YT + R],
                                    scalar1=float(B), scalar2=m1_s[:],
                                    op0=ALU.mult, op1=ALU.subtract)
            k_s = sb.tile([128, R], F32, tag="ks")
            nc.vector.tensor_scalar(out=k_s[:], in0=ysq_s[:, 0:R],
                                    scalar1=-0.5 / B, scalar2=s2c_s[:],
                                    op0=ALU.mult, op1=ALU.add)

            # ---- tail: t = mu*G ; q = t/B^2 + K ; w = q*inv ; reduce ----
            t_s = sb.tile([Y_DIM, R], F32, tag="ts")
            nc.vector.tensor_mul(t_s[:], mu_p[:], g_s[:])
            tb_s = sb.tile([Y_DIM, R], F32, tag="tbs")
            nc.vector.tensor_scalar_mul(tb_s[:], t_s[:], 1.0 / (B * B))
            q_s = sb.tile([Y_DIM, R], F32, tag="qs")
            nc.vector.tensor_add(q_s[:], tb_s[:], k_s[:])
            w_s = sb.tile([Y_DIM, R], F32, tag="ws")
            nc.vector.tensor_mul(w_s[:], q_s[:], inv_s[:])
            tot_s = sb.tile([128, 1], F32, tag="tot")
            nc.vector.tensor_reduce(out=tot_s[:], in_=w_s[:],
                                    axis=mybir.AxisListType.X, op=ALU.add)

            # ---- cross-partition reduce on PE -> [1,1] -> 4B DMA out ----
            res_p = ps.tile([1, 1], F32, tag="res")
            nc.tensor.matmul(res_p[:], tot_s[:], ones_s[:],
                             start=True, stop=True)
            res_s = sb.tile([1, 1], F32, tag="ress")
            nc.vector.tensor_copy(out=res_s[:], in_=res_p[:])
            nc.sync.dma_start(out=out_d[:], in_=res_s[:])

    nc.compile()
    return nc


def _get_nc():
    if "nc" not in _CACHE:
        _CACHE["nc"] = _build()
    return _CACHE["nc"]


def _pack_weights(w1_mu, b1_mu, w2_mu, b2_mu, w1_lv, b1_lv, w2_lv, b2_lv):
    f = np.float32
    wa = np.zeros((128, A_XT), f)
    w1m = np.asarray(w1_mu, f).reshape(XC, 128, HID)
    w1l = np.asarray(w1_lv, f).reshape(XC, 128, HID)
    for k in range(XC):
        wa[:, k * W1C:k * W1C + 8] = w1m[k]
        wa[:, k * W1C + 32:k * W1C + 40] = w1l[k]
    wa[0:8, A_B1] = np.asarray(b1_mu, f)
    wa[8, A_B1] = 1.0
    wa[32:40, A_B1] = np.asarray(b1_lv, f)
    wa[40, A_B1] = 1.0
    wb = np.zeros((128, Y_DIM), f)
    wb[0:8, :] = np.asarray(w2_mu, f)
    wb[8, :] = np.asarray(b2_mu, f)
    wb[32:40, :] = np.asarray(w2_lv, f)
    wb[40, :] = np.asarray(b2_lv, f)
    return wa, wb


def kernel(x_samples, y_samples, w1_mu, b1_mu, w2_mu, b2_mu,
           w1_lv, b1_lv, w2_lv, b2_lv, **profile_kwargs):
    from concourse import bass_utils

    f = np.float32
    wa, wb = _pack_weights(w1_mu, b1_mu, w2_mu, b2_mu,
                           w1_lv, b1_lv, w2_lv, b2_lv)
    yt = np.ascontiguousarray(np.asarray(y_samples, f).T)      # [128, 512]
    x = np.asarray(x_samples, f)

    in_maps = []
    for c in range(N_CORES):
        xt = np.ascontiguousarray(x[c * R:(c + 1) * R].T).reshape(XC, 128, R)
        ytc = np.roll(yt, -c * R, axis=1)
        b1 = np.empty((128, B1_COLS), f)
        b1[:, :A_XT] = wa
        for k in range(3):
            b1[:, A_XT + k * R:A_XT + (k + 1) * R] = xt[k]
        b1[:, A_YT:] = ytc[:, :YH]
        b2 = np.empty((128, B2_COLS), f)
        b2[:, :B_W2] = wb
        for k in range(3):
            b2[:, B_XT + k * R:B_XT + (k + 1) * R] = xt[3 + k]
        b2[:, B_YT:] = ytc[:, YH:]
        in_maps.append({"b1": b1, "b2": b2})

    nc = _get_nc()
    res = bass_utils.run_bass_kernel_spmd(
        nc, in_maps, core_ids=list(range(N_CORES)), **profile_kwargs
    )
    total = sum(float(m["out"][0, 0]) for m in res.results)
    total -= np.log1p(np.exp(-20.0) / (B - 1))
    out = np.array(total, dtype=np.float32)
    if profile_kwargs:
        return out, res
    return out


# revision 12
# speedup vs baseline: 1.3625x; 1.0243x over previous
"""Trainium2 Bass kernel for nn_L1OutUB (L1-out upper bound contrastive loss).

Math: the reference builds a [B,B,B] tensor `inpt[a,i,j] = all_probs[i,j] +
(-20 if a==i else 0)` and logsumexps over `a`.  That logsumexp is exactly
`all_probs[i,j] + log(B-1+e^-20)`, so

    result = mean(positive) - mean(all_probs) - log1p(e^-20 / (B-1))

and `sum_j all_probs[i,j]` collapses onto per-column moments of y
(S2[d] = sum_j y[j,d]^2, M1[d] = sum_j y[j,d]).  The -0.5*logvar terms
cancel exactly between positive and negative, and the per-(i,d) mu^2 terms
cancel between the positive and all-pairs branches:

    contrib[i,d] = inv[i,d] * ( mu[i,d]*(yc/B - M1/B^2) + K[i,d] )
      K   = S2/(2B^2) - yc^2/(2B)     (yc = matched y rows, feature-major)
      inv = exp(-tanh(z_lv))

Sharding: rows of x across 8 cores (64 rows each); every core gets the full
y (column-rotated so its matched rows sit at cols 0:64 of yT) and computes
the global column moments redundantly.  Host sums the 8 scalar partials
(the "all-reduce").

Layout/overlap decisions (all transposes done on host; PE does matmuls only):
  - Two input DMAs per HWDGE queue: blob1 = [w1|b1|xT chunks 0:2|yT half A],
    blob2 = [w2|xT chunks 3:5|yT half B].  x parts stream ahead of y parts;
    y moments are computed per-half as the data lands.
  - y is shipped pre-transposed (yT [128,512]) so moments are free-dim DVE
    reductions and yc/yc^2 are column slices.
  - L1 runs both nets in one 6-matmul chain ([128,41] stationaries, mu rows
    0:8, lv rows 32:40).  Bias+relu fused into one ACT op whose bias column
    also manufactures the two all-ones rows (bias[8]=bias[40]=1, relu(0+1)).
  - L2 folds its biases via those ones-rows, so mu / z_lv leave PSUM done.
  - ACT does relu/tanh/exp only (one table set, load overlaps the DMAs).
  - Final reduce: free-dim DVE reduce -> [128,1], PE matmul against a ones
    column -> [1,1] -> single 4-byte output DMA (a [128,1] output DMA costs
    ~7us in scattered-write completion; don't do that).
"""

import numpy as np

import concourse.bacc as bacc
import concourse.tile as tile
from concourse import mybir

F32 = mybir.dt.float32
AF = mybir.ActivationFunctionType
ALU = mybir.AluOpType

B, X_DIM, Y_DIM, HID = 512, 768, 128, 8
N_CORES = 8
R = B // N_CORES          # rows per core = 64
XC = X_DIM // 128         # x feature chunks = 6
YH = B // 2               # yT half width = 256

W1C = 41                  # L1 stationary cols (mu 0:8, lv 32:40, 40 = ones)
A_W1 = XC * W1C           # 246
A_B1 = A_W1               # bias column index
A_XT = A_B1 + 1           # 247
B1_COLS = A_XT + 3 * R    # blob1 width: 439 (w1|b1|xT chunks 0:2)
B_W2 = Y_DIM              # blob2: w2 block cols 0:128
B_XT = B_W2               # xT chunks 3:5 at 128:320
B2_COLS = B_XT + 3 * R    # blob2 width: 320

_CACHE = {}


def _build():
    nc = bacc.Bacc("TRN2", target_bir_lowering=False, debug=False,
                   num_devices=N_CORES)

    b1_d = nc.dram_tensor("b1", [128, B1_COLS], F32, kind="ExternalInput")
    b2_d = nc.dram_tensor("b2", [128, B2_COLS], F32, kind="ExternalInput")
    ya_d = nc.dram_tensor("ya", [128, YH], F32, kind="ExternalInput")
    yb_d = nc.dram_tensor("yb", [128, YH], F32, kind="ExternalInput")
    out_d = nc.dram_tensor("out", [1, 1], F32, kind="ExternalOutput")

    with tile.TileContext(nc) as tc:
        with (
            tc.tile_pool(name="sb", bufs=1) as sb,
            tc.tile_pool(name="ps", bufs=1, space="PSUM") as ps,
        ):
            # x-parts first on both queues so L1 never waits on y; the
            # y halves ride behind them.  SWDGE (gpsimd) carries blob2 so
            # the ACT ring stays free for its table load + activations.
            b1_s = sb.tile([128, B1_COLS], F32, tag="b1")
            nc.sync.dma_start(out=b1_s[:], in_=b1_d[:])
            b2_s = sb.tile([128, B2_COLS], F32, tag="b2")
            nc.gpsimd.dma_start(out=b2_s[:], in_=b2_d[:])
            ya_s = sb.tile([128, YH], F32, tag="ya")
            nc.sync.dma_start(out=ya_s[:], in_=ya_d[:])
            yb_s = sb.tile([128, YH], F32, tag="yb")
            nc.gpsimd.dma_start(out=yb_s[:], in_=yb_d[:])

            yA = ya_s[:]
            yB = yb_s[:]

            # ---- y column moments, one half per blob as it lands ----
            ysq_s = sb.tile([128, YH], F32, tag="ysq")   # only half A kept
            momh_s = sb.tile([128, 4], F32, tag="momh")
            nc.vector.tensor_mul(ysq_s[:], yA, yA)
            nc.vector.tensor_reduce(out=momh_s[:, 0:1], in_=ysq_s[:],
                                    axis=mybir.AxisListType.X, op=ALU.add)
            nc.vector.tensor_reduce(out=momh_s[:, 1:2], in_=yA,
                                    axis=mybir.AxisListType.X, op=ALU.add)

            # ---- MLP layer 1, both nets in one accumulation chain ----
            hb_p = ps.tile([W1C, R], F32, tag="hb")
            xt_views = [
                b1_s[:, A_XT:A_XT + R],
                b1_s[:, A_XT + R:A_XT + 2 * R],
                b1_s[:, A_XT + 2 * R:A_XT + 3 * R],
                b2_s[:, B_XT:B_XT + R],
                b2_s[:, B_XT + R:B_XT + 2 * R],
                b2_s[:, B_XT + 2 * R:B_XT + 3 * R],
            ]
            order = [3, 4, 5, 0, 1, 2]
            for i, k in enumerate(order):
                nc.tensor.matmul(hb_p[:], b1_s[:, k * W1C:(k + 1) * W1C],
                                 xt_views[k],
                                 start=(i == 0), stop=(i == len(order) - 1))

            # ---- fused bias+relu on ACT; rows 8/40 become ones-rows ----
            hb_s = sb.tile([W1C, R], F32, tag="hbs")
            nc.scalar.activation(out=hb_s[:], in_=hb_p[:], func=AF.Relu,
                                 bias=b1_s[0:W1C, A_B1:A_B1 + 1])

            # ---- second-half moments (may land after L1 kicks off) ----
            ysqB_s = sb.tile([128, YH], F32, tag="ysqB")
            nc.vector.tensor_mul(ysqB_s[:], yB, yB)
            nc.vector.tensor_reduce(out=momh_s[:, 2:3], in_=ysqB_s[:],
                                    axis=mybir.AxisListType.X, op=ALU.add)
            nc.vector.tensor_reduce(out=momh_s[:, 3:4], in_=yB,
                                    axis=mybir.AxisListType.X, op=ALU.add)

            # ---- MLP layer 2 (bias via ones-rows): mu, z_lv in PSUM ----
            mu_p = ps.tile([Y_DIM, R], F32, tag="mup")
            lv_p = ps.tile([Y_DIM, R], F32, tag="lvp")
            nc.tensor.matmul(mu_p[:], b2_s[0:9, 0:Y_DIM], hb_s[0:9, :],
                             start=True, stop=True)
            nc.tensor.matmul(lv_p[:], b2_s[32:41, 0:Y_DIM], hb_s[32:41, :],
                             start=True, stop=True)

            # ---- inv = exp(-tanh(z_lv)) on ACT ----
            lv_s = sb.tile([Y_DIM, R], F32, tag="lvs")
            nc.scalar.activation(out=lv_s[:], in_=lv_p[:], func=AF.Tanh)
            inv_s = sb.tile([Y_DIM, R], F32, tag="invs")
            nc.scalar.activation(out=inv_s[:], in_=lv_s[:], func=AF.Exp,
                                 scale=-1.0)

            # ---- combine half-moments; G = yc*B - M1 ; K from ysq ----
            m1_s = sb.tile([128, 1], F32, tag="m1")
            nc.vector.tensor_scalar(out=m1_s[:], in0=momh_s[:, 1:2],
                                    scalar1=momh_s[:, 3:4],
                                    scalar2=1.0 / (B * B),
                                    op0=ALU.add, op1=ALU.mult)
            s2c_s = sb.tile([128, 1], F32, tag="s2c")
            nc.vector.tensor_scalar(out=s2c_s[:], in0=momh_s[:, 0:1],
                                    scalar1=momh_s[:, 2:3],
                                    scalar2=0.5 / (B * B),
                                    op0=ALU.add, op1=ALU.mult)
            g_s = sb.tile([128, R], F32, tag="gs")
            nc.vector.tensor_scalar(out=g_s[:], in0=ya_s[:, 0:R],
                                    scalar1=1.0 / B, scalar2=m1_s[:],
                                    op0=ALU.mult, op1=ALU.subtract)
            k_s = sb.tile([128, R], F32, tag="ks")
            nc.vector.tensor_scalar(out=k_s[:], in0=ysq_s[:, 0:R],
                                    scalar1=-0.5 / B, scalar2=s2c_s[:],
                                    op0=ALU.mult, op1=ALU.add)

            # ---- tail: t = mu*G ; q = t/B^2 + K ; w = q*inv ; reduce ----
            t_s = sb.tile([Y_DIM, R], F32, tag="ts")
            nc.vector.tensor_mul(t_s[:], mu_p[:], g_s[:])
            q_s = sb.tile([Y_DIM, R], F32, tag="qs")
            nc.vector.tensor_add(q_s[:], t_s[:], k_s[:])
            w_s = sb.tile([Y_DIM, R], F32, tag="ws")
            nc.vector.tensor_mul(w_s[:], q_s[:], inv_s[:])
            tot_s = sb.tile([128, 1], F32, tag="tot")
            nc.vector.tensor_reduce(out=tot_s[:], in_=w_s[:],
                                    axis=mybir.AxisListType.X, op=ALU.add)

            # ---- cross-partition reduce on PE -> [1,1] -> 4B DMA out ----
            ones_ap = nc.const_aps.aps[(F32, 1.0)]
            res_p = ps.tile([1, 1], F32, tag="res")
            nc.tensor.matmul(res_p[:], tot_s[:], ones_ap,
                             start=True, stop=True)
            res_s = sb.tile([1, 1], F32, tag="ress")
            nc.vector.tensor_copy(out=res_s[:], in_=res_p[:])
            nc.sync.dma_start(out=out_d[:], in_=res_s[:])

    nc.compile()
    return nc


def _get_nc():
    if "nc" not in _CACHE:
        _CACHE["nc"] = _build()
    return _CACHE["nc"]


def _pack_weights(w1_mu, b1_mu, w2_mu, b2_mu, w1_lv, b1_lv, w2_lv, b2_lv):
    f = np.float32
    wa = np.zeros((128, A_XT), f)
    w1m = np.asarray(w1_mu, f).reshape(XC, 128, HID)
    w1l = np.asarray(w1_lv, f).reshape(XC, 128, HID)
    for k in range(XC):
        wa[:, k * W1C:k * W1C + 8] = w1m[k]
        wa[:, k * W1C + 32:k * W1C + 40] = w1l[k]
    wa[0:8, A_B1] = np.asarray(b1_mu, f)
    wa[8, A_B1] = 1.0
    wa[32:40, A_B1] = np.asarray(b1_lv, f)
    wa[40, A_B1] = 1.0
    wb = np.zeros((128, Y_DIM), f)
    wb[0:8, :] = np.asarray(w2_mu, f)
    wb[8, :] = np.asarray(b2_mu, f)
    wb[32:40, :] = np.asarray(w2_lv, f)
    wb[40, :] = np.asarray(b2_lv, f)
    return wa, wb


def kernel(x_samples, y_samples, w1_mu, b1_mu, w2_mu, b2_mu,
           w1_lv, b1_lv, w2_lv, b2_lv, **profile_kwargs):
    from concourse import bass_utils

    f = np.float32
    wa, wb = _pack_weights(w1_mu, b1_mu, w2_mu, b2_mu,
                           w1_lv, b1_lv, w2_lv, b2_lv)
    yt = np.ascontiguousarray(np.asarray(y_samples, f).T)      # [128, 512]
    x = np.asarray(x_samples, f)

    in_maps = []
    for c in range(N_CORES):
        xt = np.ascontiguousarray(x[c * R:(c + 1) * R].T).reshape(XC, 128, R)
        ytc = np.roll(yt, -c * R, axis=1)
        b1 = np.empty((128, B1_COLS), f)
        b1[:, :A_XT] = wa
        for k in range(3):
            b1[:, A_XT + k * R:A_XT + (k + 1) * R] = xt[k]
        b2 = np.empty((128, B2_COLS), f)
        b2[:, :B_W2] = wb
        for k in range(3):
            b2[:, B_XT + k * R:B_XT + (k + 1) * R] = xt[3 + k]
        in_maps.append({"b1": b1, "b2": b2,
                        "ya": np.ascontiguousarray(ytc[:, :YH]),
                        "yb": np.ascontiguousarray(ytc[:, YH:])})

    nc = _get_nc()
    res = bass_utils.run_bass_kernel_spmd(
        nc, in_maps, core_ids=list(range(N_CORES)), **profile_kwargs
    )
    total = sum(float(m["out"][0, 0]) for m in res.results)
    total -= np.log1p(np.exp(-20.0) / (B - 1))
    out = np.array(total, dtype=np.float32)
    if profile_kwargs:
        return out, res
    return out


# revision 13
# speedup vs baseline: 1.3955x; 1.0242x over previous
"""Trainium2 Bass kernel for nn_L1OutUB (L1-out upper bound contrastive loss).

Math: the reference builds a [B,B,B] tensor `inpt[a,i,j] = all_probs[i,j] +
(-20 if a==i else 0)` and logsumexps over `a`.  That logsumexp is exactly
`all_probs[i,j] + log(B-1+e^-20)`, so

    result = mean(positive) - mean(all_probs) - log1p(e^-20 / (B-1))

and `sum_j all_probs[i,j]` collapses onto per-column moments of y
(S2[d] = sum_j y[j,d]^2, M1[d] = sum_j y[j,d]).  The -0.5*logvar terms
cancel exactly between positive and negative, and the per-(i,d) mu^2 terms
cancel between the positive and all-pairs branches:

    contrib[i,d] = inv[i,d] * ( mu[i,d]*(yc/B - M1/B^2) + K[i,d] )
      K   = S2/(2B^2) - yc^2/(2B)     (yc = matched y rows, feature-major)
      inv = exp(-tanh(z_lv))

Sharding: rows of x across 8 cores (64 rows each); every core gets the full
y (column-rotated so its matched rows sit at cols 0:64 of yT) and computes
the global column moments redundantly.  Host sums the 8 scalar partials
(the "all-reduce").

Layout/overlap decisions (all transposes done on host; PE does matmuls only):
  - Two input DMAs per HWDGE queue: blob1 = [w1|b1|xT chunks 0:2|yT half A],
    blob2 = [w2|xT chunks 3:5|yT half B].  x parts stream ahead of y parts;
    y moments are computed per-half as the data lands.
  - y is shipped pre-transposed (yT [128,512]) so moments are free-dim DVE
    reductions and yc/yc^2 are column slices.
  - L1 runs both nets in one 6-matmul chain ([128,41] stationaries, mu rows
    0:8, lv rows 32:40).  Bias+relu fused into one ACT op whose bias column
    also manufactures the two all-ones rows (bias[8]=bias[40]=1, relu(0+1)).
  - L2 folds its biases via those ones-rows, so mu / z_lv leave PSUM done.
  - ACT does relu/tanh/exp only (one table set, load overlaps the DMAs).
  - Final reduce: free-dim DVE reduce -> [128,1], PE matmul against a ones
    column -> [1,1] -> single 4-byte output DMA (a [128,1] output DMA costs
    ~7us in scattered-write completion; don't do that).
"""

import numpy as np

import concourse.bacc as bacc
import concourse.tile as tile
from concourse import mybir

F32 = mybir.dt.float32
AF = mybir.ActivationFunctionType
ALU = mybir.AluOpType

B, X_DIM, Y_DIM, HID = 512, 768, 128, 8
N_CORES = 8
R = B // N_CORES          # rows per core = 64
XC = X_DIM // 128         # x feature chunks = 6
YH = B // 2               # yT half width = 256

W1C = 41                  # L1 stationary cols (mu 0:8, lv 32:40, 40 = ones)
A_W1 = XC * W1C           # 246
A_B1 = A_W1               # bias column index
A_XT = A_B1 + 1           # 247
B1_COLS = A_XT + 3 * R    # blob1 width: 439 (w1|b1|xT chunks 0:2)
B_W2 = Y_DIM              # blob2: w2 block cols 0:128
B_XT = B_W2               # xT chunks 3:5 at 128:320
B2_COLS = B_XT + 3 * R    # blob2 width: 320

_CACHE = {}


def _build():
    nc = bacc.Bacc("TRN2", target_bir_lowering=False, debug=False,
                   num_devices=N_CORES)

    b1_d = nc.dram_tensor("b1", [128, B1_COLS], F32, kind="ExternalInput")
    b2_d = nc.dram_tensor("b2", [128, B2_COLS], F32, kind="ExternalInput")
    ya_d = nc.dram_tensor("ya", [128, YH], F32, kind="ExternalInput")
    yb_d = nc.dram_tensor("yb", [128, YH], F32, kind="ExternalInput")
    out_d = nc.dram_tensor("out", [1, 1], F32, kind="ExternalOutput")

    with tile.TileContext(nc) as tc:
        with (
            tc.tile_pool(name="sb", bufs=1) as sb,
            tc.tile_pool(name="ps", bufs=1, space="PSUM") as ps,
        ):
            # x-parts first on both queues so L1 never waits on y; the
            # y halves ride behind them.  SWDGE (gpsimd) carries blob2 so
            # the ACT ring stays free for its table load + activations.
            dum_s = sb.tile([128, 1], F32, tag="dum")
            nc.scalar.activation(out=dum_s[:], in_=nc.const_aps.aps[(F32, 0.0)],
                                 func=AF.Tanh)

            b1_s = sb.tile([128, B1_COLS], F32, tag="b1")
            nc.sync.dma_start(out=b1_s[:], in_=b1_d[:])
            b2_s = sb.tile([128, B2_COLS], F32, tag="b2")
            nc.gpsimd.dma_start(out=b2_s[:], in_=b2_d[:])
            ya_s = sb.tile([128, YH], F32, tag="ya")
            nc.sync.dma_start(out=ya_s[:], in_=ya_d[:])
            yb_s = sb.tile([128, YH], F32, tag="yb")
            nc.gpsimd.dma_start(out=yb_s[:], in_=yb_d[:])

            yA = ya_s[:]
            yB = yb_s[:]

            # ---- y column moments: squares+S2 fused on ACT (idle early),
            # M1 reductions on DVE ----
            ysq_s = sb.tile([128, YH], F32, tag="ysq")   # ya^2; cols 0:64 = yc^2
            ysqB_s = sb.tile([128, YH], F32, tag="ysqB")
            momh_s = sb.tile([128, 4], F32, tag="momh")
            nc.scalar.activation(out=ysq_s[:], in_=yA, func=AF.Square,
                                 accum_out=momh_s[:, 0:1])
            nc.scalar.activation(out=ysqB_s[:], in_=yB, func=AF.Square,
                                 accum_out=momh_s[:, 2:3])
            nc.vector.tensor_reduce(out=momh_s[:, 1:2], in_=yA,
                                    axis=mybir.AxisListType.X, op=ALU.add)
            nc.vector.tensor_reduce(out=momh_s[:, 3:4], in_=yB,
                                    axis=mybir.AxisListType.X, op=ALU.add)

            # ---- MLP layer 1, both nets in one accumulation chain ----
            hb_p = ps.tile([W1C, R], F32, tag="hb")
            xt_views = [
                b1_s[:, A_XT:A_XT + R],
                b1_s[:, A_XT + R:A_XT + 2 * R],
                b1_s[:, A_XT + 2 * R:A_XT + 3 * R],
                b2_s[:, B_XT:B_XT + R],
                b2_s[:, B_XT + R:B_XT + 2 * R],
                b2_s[:, B_XT + 2 * R:B_XT + 3 * R],
            ]
            order = [0, 1, 2, 3, 4, 5]
            for i, k in enumerate(order):
                nc.tensor.matmul(hb_p[:], b1_s[:, k * W1C:(k + 1) * W1C],
                                 xt_views[k],
                                 start=(i == 0), stop=(i == len(order) - 1))

            # ---- fused bias+relu on ACT; rows 8/40 become ones-rows ----
            hb_s = sb.tile([W1C, R], F32, tag="hbs")
            nc.scalar.activation(out=hb_s[:], in_=hb_p[:], func=AF.Relu,
                                 bias=b1_s[0:W1C, A_B1:A_B1 + 1])

            # ---- MLP layer 2 (bias via ones-rows): mu, z_lv in PSUM ----
            mu_p = ps.tile([Y_DIM, R], F32, tag="mup")
            lv_p = ps.tile([Y_DIM, R], F32, tag="lvp")
            nc.tensor.matmul(mu_p[:], b2_s[0:9, 0:Y_DIM], hb_s[0:9, :],
                             start=True, stop=True)
            nc.tensor.matmul(lv_p[:], b2_s[32:41, 0:Y_DIM], hb_s[32:41, :],
                             start=True, stop=True)

            # ---- inv = exp(-tanh(z_lv)) on ACT ----
            lv_s = sb.tile([Y_DIM, R], F32, tag="lvs")
            nc.scalar.activation(out=lv_s[:], in_=lv_p[:], func=AF.Tanh)
            inv_s = sb.tile([Y_DIM, R], F32, tag="invs")
            nc.scalar.activation(out=inv_s[:], in_=lv_s[:], func=AF.Exp,
                                 scale=-1.0)

            # ---- combine half-moments; G = yc*B - M1 ; K from ysq ----
            m1_s = sb.tile([128, 1], F32, tag="m1")
            nc.vector.tensor_scalar(out=m1_s[:], in0=momh_s[:, 1:2],
                                    scalar1=momh_s[:, 3:4],
                                    scalar2=1.0 / (B * B),
                                    op0=ALU.add, op1=ALU.mult)
            s2c_s = sb.tile([128, 1], F32, tag="s2c")
            nc.vector.tensor_scalar(out=s2c_s[:], in0=momh_s[:, 0:1],
                                    scalar1=momh_s[:, 2:3],
                                    scalar2=0.5 / (B * B),
                                    op0=ALU.add, op1=ALU.mult)
            g_s = sb.tile([128, R], F32, tag="gs")
            nc.vector.tensor_scalar(out=g_s[:], in0=ya_s[:, 0:R],
                                    scalar1=1.0 / B, scalar2=m1_s[:],
                                    op0=ALU.mult, op1=ALU.subtract)
            k_s = sb.tile([128, R], F32, tag="ks")
            nc.vector.tensor_scalar(out=k_s[:], in0=ysq_s[:, 0:R],
                                    scalar1=-0.5 / B, scalar2=s2c_s[:],
                                    op0=ALU.mult, op1=ALU.add)

            # ---- tail: t = mu*G ; q = t/B^2 + K ; w = q*inv ; reduce ----
            t_s = sb.tile([Y_DIM, R], F32, tag="ts")
            nc.vector.tensor_mul(t_s[:], mu_p[:], g_s[:])
            q_s = sb.tile([Y_DIM, R], F32, tag="qs")
            nc.vector.tensor_add(q_s[:], t_s[:], k_s[:])
            w_s = sb.tile([Y_DIM, R], F32, tag="ws")
            nc.vector.tensor_mul(w_s[:], q_s[:], inv_s[:])
            tot_s = sb.tile([128, 1], F32, tag="tot")
            nc.vector.tensor_reduce(out=tot_s[:], in_=w_s[:],
                                    axis=mybir.AxisListType.X, op=ALU.add)

            # ---- cross-partition reduce on PE -> [1,1] -> 4B DMA out ----
            ones_ap = nc.const_aps.aps[(F32, 1.0)]
            res_p = ps.tile([1, 1], F32, tag="res")
            nc.tensor.matmul(res_p[:], tot_s[:], ones_ap,
                             start=True, stop=True)
            res_s = sb.tile([1, 1], F32, tag="ress")
            nc.vector.tensor_copy(out=res_s[:], in_=res_p[:])
            nc.sync.dma_start(out=out_d[:], in_=res_s[:])

    nc.compile()
    return nc


def _get_nc():
    if "nc" not in _CACHE:
        _CACHE["nc"] = _build()
    return _CACHE["nc"]


def _pack_weights(w1_mu, b1_mu, w2_mu, b2_mu, w1_lv, b1_lv, w2_lv, b2_lv):
    f = np.float32
    wa = np.zeros((128, A_XT), f)
    w1m = np.asarray(w1_mu, f).reshape(XC, 128, HID)
    w1l = np.asarray(w1_lv, f).reshape(XC, 128, HID)
    for k in range(XC):
        wa[:, k * W1C:k * W1C + 8] = w1m[k]
        wa[:, k * W1C + 32:k * W1C + 40] = w1l[k]
    wa[0:8, A_B1] = np.asarray(b1_mu, f)
    wa[8, A_B1] = 1.0
    wa[32:40, A_B1] = np.asarray(b1_lv, f)
    wa[40, A_B1] = 1.0
    wb = np.zeros((128, Y_DIM), f)
    wb[0:8, :] = np.asarray(w2_mu, f)
    wb[8, :] = np.asarray(b2_mu, f)
    wb[32:40, :] = np.asarray(w2_lv, f)
    wb[40, :] = np.asarray(b2_lv, f)
    return wa, wb


def kernel(x_samples, y_samples, w1_mu, b1_mu, w2_mu, b2_mu,
           w1_lv, b1_lv, w2_lv, b2_lv, **profile_kwargs):
    from concourse import bass_utils

    f = np.float32
    wa, wb = _pack_weights(w1_mu, b1_mu, w2_mu, b2_mu,
                           w1_lv, b1_lv, w2_lv, b2_lv)
    yt = np.ascontiguousarray(np.asarray(y_samples, f).T)      # [128, 512]
    x = np.asarray(x_samples, f)

    in_maps = []
    for c in range(N_CORES):
        xt = np.ascontiguousarray(x[c * R:(c + 1) * R].T).reshape(XC, 128, R)
        ytc = np.roll(yt, -c * R, axis=1)
        b1 = np.empty((128, B1_COLS), f)
        b1[:, :A_XT] = wa
        for k in range(3):
            b1[:, A_XT + k * R:A_XT + (k + 1) * R] = xt[k]
        b2 = np.empty((128, B2_COLS), f)
        b2[:, :B_W2] = wb
        for k in range(3):
            b2[:, B_XT + k * R:B_XT + (k + 1) * R] = xt[3 + k]
        in_maps.append({"b1": b1, "b2": b2,
                        "ya": np.ascontiguousarray(ytc[:, :YH]),
                        "yb": np.ascontiguousarray(ytc[:, YH:])})

    nc = _get_nc()
    res = bass_utils.run_bass_kernel_spmd(
        nc, in_maps, core_ids=list(range(N_CORES)), **profile_kwargs
    )
    total = sum(float(m["out"][0, 0]) for m in res.results)
    total -= np.log1p(np.exp(-20.0) / (B - 1))
    out = np.array(total, dtype=np.float32)
    if profile_kwargs:
        return out, res
    return out


# revision 15
# speedup vs baseline: 1.4210x; 1.0183x over previous
"""Trainium2 Bass kernel for nn_L1OutUB (L1-out upper bound contrastive loss).

Math: the reference builds a [B,B,B] tensor `inpt[a,i,j] = all_probs[i,j] +
(-20 if a==i else 0)` and logsumexps over `a`.  That logsumexp is exactly
`all_probs[i,j] + log(B-1+e^-20)`, so

    result = mean(positive) - mean(all_probs) - log1p(e^-20 / (B-1))

and `sum_j all_probs[i,j]` collapses onto per-column moments of y
(S2[d] = sum_j y[j,d]^2, M1[d] = sum_j y[j,d]).  The -0.5*logvar terms
cancel exactly between positive and negative, and the per-(i,d) mu^2 terms
cancel between the positive and all-pairs branches:

    contrib[i,d] = inv[i,d] * ( mu[i,d]*(yc/B - M1/B^2) + K[i,d] )
      K   = S2/(2B^2) - yc^2/(2B)     (yc = matched y rows, feature-major)
      inv = exp(-tanh(z_lv))

Sharding: rows of x across 8 cores (64 rows each); every core gets the full
y (column-rotated so its matched rows sit at cols 0:64 of yT) and computes
the global column moments redundantly.  Host sums the 8 scalar partials
(the "all-reduce").

Layout/overlap decisions (all transposes done on host; PE does matmuls only):
  - Two input DMAs per HWDGE queue: blob1 = [w1|b1|xT chunks 0:2|yT half A],
    blob2 = [w2|xT chunks 3:5|yT half B].  x parts stream ahead of y parts;
    y moments are computed per-half as the data lands.
  - y is shipped pre-transposed (yT [128,512]) so moments are free-dim DVE
    reductions and yc/yc^2 are column slices.
  - L1 runs both nets in one 6-matmul chain ([128,41] stationaries, mu rows
    0:8, lv rows 32:40).  Bias+relu fused into one ACT op whose bias column
    also manufactures the two all-ones rows (bias[8]=bias[40]=1, relu(0+1)).
  - L2 folds its biases via those ones-rows, so mu / z_lv leave PSUM done.
  - ACT does relu/tanh/exp only (one table set, load overlaps the DMAs).
  - Final reduce: free-dim DVE reduce -> [128,1], PE matmul against a ones
    column -> [1,1] -> single 4-byte output DMA (a [128,1] output DMA costs
    ~7us in scattered-write completion; don't do that).
"""

import numpy as np

import concourse.bacc as bacc
import concourse.tile as tile
from concourse import mybir

F32 = mybir.dt.float32
AF = mybir.ActivationFunctionType
ALU = mybir.AluOpType

B, X_DIM, Y_DIM, HID = 512, 768, 128, 8
N_CORES = 8
R = B // N_CORES          # rows per core = 64
XC = X_DIM // 128         # x feature chunks = 6
YH = B // 2               # yT half width = 256

W1C = 41                  # L1 stationary cols (mu 0:8, lv 32:40, 40 = ones)
A_W1 = XC * W1C           # 246
A_B1 = A_W1               # bias column index
A_XT = A_B1 + 1           # 247
B1_COLS = A_XT + 3 * R    # blob1 width: 439 (w1|b1|xT chunks 0:2)
B_W2 = Y_DIM              # blob2: w2 block cols 0:128
B_XT = B_W2               # xT chunks 3:5 at 128:320
B2_COLS = B_XT + 3 * R    # blob2 width: 320

_CACHE = {}


def _build():
    nc = bacc.Bacc("TRN2", target_bir_lowering=False, debug=False,
                   num_devices=N_CORES)

    b1_d = nc.dram_tensor("b1", [128, B1_COLS], F32, kind="ExternalInput")
    b2_d = nc.dram_tensor("b2", [128, B2_COLS], F32, kind="ExternalInput")
    ya_d = nc.dram_tensor("ya", [128, YH], F32, kind="ExternalInput")
    yb_d = nc.dram_tensor("yb", [128, YH], F32, kind="ExternalInput")
    out_d = nc.dram_tensor("out", [1, 1], F32, kind="ExternalOutput")

    with tile.TileContext(nc) as tc:
        with (
            tc.tile_pool(name="sb", bufs=1) as sb,
            tc.tile_pool(name="ps", bufs=1, space="PSUM") as ps,
        ):
            # x-parts first on both queues so L1 never waits on y; the
            # y halves ride behind them.  SWDGE (gpsimd) carries blob2 so
            # the ACT ring stays free for its table load + activations.
            dum_s = sb.tile([128, 1], F32, tag="dum")
            nc.scalar.activation(out=dum_s[:], in_=nc.const_aps.aps[(F32, 0.0)],
                                 func=AF.Tanh)

            b1_s = sb.tile([128, B1_COLS], F32, tag="b1")
            nc.sync.dma_start(out=b1_s[:], in_=b1_d[:])
            b2_s = sb.tile([128, B2_COLS], F32, tag="b2")
            nc.gpsimd.dma_start(out=b2_s[:], in_=b2_d[:])
            ya_s = sb.tile([128, YH], F32, tag="ya")
            nc.sync.dma_start(out=ya_s[:], in_=ya_d[:])
            yb_s = sb.tile([128, YH], F32, tag="yb")
            nc.gpsimd.dma_start(out=yb_s[:], in_=yb_d[:])

            yA = ya_s[:]
            yB = yb_s[:]

            # PE sits idle ~3.5us while inputs stream; run dummy matmuls so
            # the HAM clock-gate is at 8/8 when the real chain starts.
            wu_s = sb.tile([128, 128], F32, tag="wu")
            nc.vector.memset(wu_s[:], 0.0)
            wu_p = ps.tile([128, 128], F32, tag="wup")
            for _ in range(8):
                nc.tensor.matmul(wu_p[:], wu_s[:], wu_s[:],
                                 start=True, stop=True)

            # ---- y column moments: half A now (square+S2 fused on ACT,
            # M1 on DVE); half B is emitted after relu so the relu slot on
            # ACT isn't blocked behind it.
            ysq_s = sb.tile([128, YH], F32, tag="ysq")   # ya^2; 0:64 = yc^2
            ysqB_s = sb.tile([128, YH], F32, tag="ysqB")
            momh_s = sb.tile([128, 4], F32, tag="momh")
            nc.vector.tensor_reduce(out=momh_s[:, 1:2], in_=yA,
                                    axis=mybir.AxisListType.X, op=ALU.add)
            nc.scalar.activation(out=ysq_s[:], in_=yA, func=AF.Square,
                                 accum_out=momh_s[:, 0:1])

            # ---- MLP layer 1, both nets in one accumulation chain ----
            hb_p = ps.tile([W1C, R], F32, tag="hb")
            xt_views = [
                b1_s[:, A_XT:A_XT + R],
                b1_s[:, A_XT + R:A_XT + 2 * R],
                b1_s[:, A_XT + 2 * R:A_XT + 3 * R],
                b2_s[:, B_XT:B_XT + R],
                b2_s[:, B_XT + R:B_XT + 2 * R],
                b2_s[:, B_XT + 2 * R:B_XT + 3 * R],
            ]
            order = [0, 1, 2, 3, 4, 5]
            for i, k in enumerate(order):
                nc.tensor.matmul(hb_p[:], b1_s[:, k * W1C:(k + 1) * W1C],
                                 xt_views[k],
                                 start=(i == 0), stop=(i == len(order) - 1))

            # ---- fused bias+relu on ACT; rows 8/40 become ones-rows ----
            hb_s = sb.tile([W1C, R], F32, tag="hbs")
            nc.scalar.activation(out=hb_s[:], in_=hb_p[:], func=AF.Relu,
                                 bias=b1_s[0:W1C, A_B1:A_B1 + 1])

            # ---- second-half moments (ACT slot right after relu) ----
            nc.vector.tensor_reduce(out=momh_s[:, 3:4], in_=yB,
                                    axis=mybir.AxisListType.X, op=ALU.add)
            nc.scalar.activation(out=ysqB_s[:], in_=yB, func=AF.Square,
                                 accum_out=momh_s[:, 2:3])

            # ---- MLP layer 2 (bias via ones-rows): mu, z_lv in PSUM ----
            mu_p = ps.tile([Y_DIM, R], F32, tag="mup")
            lv_p = ps.tile([Y_DIM, R], F32, tag="lvp")
            nc.tensor.matmul(mu_p[:], b2_s[0:9, 0:Y_DIM], hb_s[0:9, :],
                             start=True, stop=True)
            nc.tensor.matmul(lv_p[:], b2_s[32:41, 0:Y_DIM], hb_s[32:41, :],
                             start=True, stop=True)

            # ---- inv = exp(-tanh(z_lv)) on ACT ----
            lv_s = sb.tile([Y_DIM, R], F32, tag="lvs")
            nc.scalar.activation(out=lv_s[:], in_=lv_p[:], func=AF.Tanh)
            inv_s = sb.tile([Y_DIM, R], F32, tag="invs")
            nc.scalar.activation(out=inv_s[:], in_=lv_s[:], func=AF.Exp,
                                 scale=-1.0)

            # ---- combine half-moments; G = yc*B - M1 ; K from ysq ----
            s2c_s = sb.tile([128, 1], F32, tag="s2c")
            nc.vector.tensor_scalar(out=s2c_s[:], in0=momh_s[:, 0:1],
                                    scalar1=momh_s[:, 2:3],
                                    scalar2=0.5 / (B * B),
                                    op0=ALU.add, op1=ALU.mult)
            m1_s = sb.tile([128, 1], F32, tag="m1")
            nc.vector.tensor_scalar(out=m1_s[:], in0=momh_s[:, 1:2],
                                    scalar1=momh_s[:, 3:4],
                                    scalar2=1.0 / (B * B),
                                    op0=ALU.add, op1=ALU.mult)

            g_s = sb.tile([128, R], F32, tag="gs")
            nc.vector.tensor_scalar(out=g_s[:], in0=ya_s[:, 0:R],
                                    scalar1=1.0 / B, scalar2=m1_s[:],
                                    op0=ALU.mult, op1=ALU.subtract)
            k_s = sb.tile([128, R], F32, tag="ks")
            nc.vector.tensor_scalar(out=k_s[:], in0=ysq_s[:, 0:R],
                                    scalar1=-0.5 / B, scalar2=s2c_s[:],
                                    op0=ALU.mult, op1=ALU.add)

            # ---- tail: t = mu*G ; q = t/B^2 + K ; w = q*inv ; reduce ----
            t_s = sb.tile([Y_DIM, R], F32, tag="ts")
            nc.vector.tensor_mul(t_s[:], mu_p[:], g_s[:])
            q_s = sb.tile([Y_DIM, R], F32, tag="qs")
            nc.vector.tensor_add(q_s[:], t_s[:], k_s[:])
            w_s = sb.tile([Y_DIM, R], F32, tag="ws")
            nc.vector.tensor_mul(w_s[:], q_s[:], inv_s[:])
            tot_s = sb.tile([128, 1], F32, tag="tot")
            nc.vector.tensor_reduce(out=tot_s[:], in_=w_s[:],
                                    axis=mybir.AxisListType.X, op=ALU.add)

            # ---- cross-partition reduce on PE -> [1,1] -> 4B DMA out ----
            ones_ap = nc.const_aps.aps[(F32, 1.0)]
            res_p = ps.tile([1, 1], F32, tag="res")
            nc.tensor.matmul(res_p[:], tot_s[:], ones_ap,
                             start=True, stop=True)
            res_s = sb.tile([1, 1], F32, tag="ress")
            nc.vector.tensor_copy(out=res_s[:], in_=res_p[:])
            nc.sync.dma_start(out=out_d[:], in_=res_s[:])

    nc.compile()
    return nc


def _get_nc():
    if "nc" not in _CACHE:
        _CACHE["nc"] = _build()
    return _CACHE["nc"]


def _pack_weights(w1_mu, b1_mu, w2_mu, b2_mu, w1_lv, b1_lv, w2_lv, b2_lv):
    f = np.float32
    wa = np.zeros((128, A_XT), f)
    w1m = np.asarray(w1_mu, f).reshape(XC, 128, HID)
    w1l = np.asarray(w1_lv, f).reshape(XC, 128, HID)
    for k in range(XC):
        wa[:, k * W1C:k * W1C + 8] = w1m[k]
        wa[:, k * W1C + 32:k * W1C + 40] = w1l[k]
    wa[0:8, A_B1] = np.asarray(b1_mu, f)
    wa[8, A_B1] = 1.0
    wa[32:40, A_B1] = np.asarray(b1_lv, f)
    wa[40, A_B1] = 1.0
    wb = np.zeros((128, Y_DIM), f)
    wb[0:8, :] = np.asarray(w2_mu, f)
    wb[8, :] = np.asarray(b2_mu, f)
    wb[32:40, :] = np.asarray(w2_lv, f)
    wb[40, :] = np.asarray(b2_lv, f)
    return wa, wb


def kernel(x_samples, y_samples, w1_mu, b1_mu, w2_mu, b2_mu,
           w1_lv, b1_lv, w2_lv, b2_lv, **profile_kwargs):
    from concourse import bass_utils

    f = np.float32
    wa, wb = _pack_weights(w1_mu, b1_mu, w2_mu, b2_mu,
                           w1_lv, b1_lv, w2_lv, b2_lv)
    yt = np.ascontiguousarray(np.asarray(y_samples, f).T)      # [128, 512]
    x = np.asarray(x_samples, f)

    in_maps = []
    for c in range(N_CORES):
        xt = np.ascontiguousarray(x[c * R:(c + 1) * R].T).reshape(XC, 128, R)
        ytc = np.roll(yt, -c * R, axis=1)
        b1 = np.empty((128, B1_COLS), f)
        b1[:, :A_XT] = wa
        for k in range(3):
            b1[:, A_XT + k * R:A_XT + (k + 1) * R] = xt[k]
        b2 = np.empty((128, B2_COLS), f)
        b2[:, :B_W2] = wb
        for k in range(3):
            b2[:, B_XT + k * R:B_XT + (k + 1) * R] = xt[3 + k]
        in_maps.append({"b1": b1, "b2": b2,
                        "ya": np.ascontiguousarray(ytc[:, :YH]),
                        "yb": np.ascontiguousarray(ytc[:, YH:])})

    nc = _get_nc()
    res = bass_utils.run_bass_kernel_spmd(
        nc, in_maps, core_ids=list(range(N_CORES)), **profile_kwargs
    )
    total = sum(float(m["out"][0, 0]) for m in res.results)
    total -= np.log1p(np.exp(-20.0) / (B - 1))
    out = np.array(total, dtype=np.float32)
    if profile_kwargs:
        return out, res
    return out
